# revision 17
# baseline (speedup 1.0000x reference)
"""GraphSAGE edge layer on 8 Trainium2 NeuronCores via Bass/Tile.  v4.3.

Gather-free, feature-major design (device cannot gather rows by edge on this
image): the HOST replicates edge endpoints' raw features into dense slot
streams (sharding/halo distribution on the host, all model FLOPs on device).

v4.3 design notes (driven by v3/v4.2 traces):
  - Slot layout: per-core nodes are degree-sorted; rank r gets
    Kr[r] = max over cores of the r-th sorted degree (shared across cores ->
    one SPMD program; ~1% padding).  Equal-k runs are made even-length by
    bumping one node into the previous (larger-k) run; an optional ghost
    node keeps the tail even.
  - Within each sub-run the slots are SLOT-MAJOR ([k, n]: slot j of all n
    nodes contiguous), so the segment-max tree folds are flat contiguous
    tensor_tensor max ops that hit the DVE 2x packed mode, and the final
    fold writes ct directly.
  - Gate pipeline per 512-slot chunk, weights grouped [z z z z][a a a a]:
      z-psum pair [128,1024] <- 2 DoubleRow fp8 matmuls (contract 256)
      a-psum [128,512]       <- fp8 matmul of 8*A_w against stride-2 src
      s = ACT sigmoid(zpair/8 + 2*B_b) over the 1024-col pair
      g = (a + 8*A_b) * s in place over s: DVE stt (1x, PSUM-bound), with a
        fraction of chunks evacuated by ACT (Identity+bias) so the DVE
        multiply runs as 2x tensor_tensor (EVAC_ACT tunable).
  - Update per 128-node block: one PSUM tile accumulates h@U1 + ct.T@(U2/8)
    + ones.T@U_b (3 matmuls); ACT Relu -> bu_r; ACT Square+accum -> ssq.
    After all blocks: one Sqrt + reciprocal, then per block a single DVE
    stt: out = bu_r * rsqrt(ssq) + h (bf16).  GPSIMD is not used: it cannot
    read PSUM and measured ~7x slower than modeled on tensor ops.
  - hloc/out shipped as bf16 (residual precision ~0.4%, gate 2e-2).

Precision: ~2e-3 rel err end-to-end vs the 2e-2 gate.
"""

import numpy as np
import ml_dtypes

from concourse import bass, bacc, mybir
from concourse.tile import TileContext
from concourse.bass_utils import run_bass_kernel_spmd

BF16 = ml_dtypes.bfloat16
F8 = ml_dtypes.float8_e4m3

N = 50000
E = 800000
D = 128
NCORES = 8
NLOC = N // NCORES          # 6250 nodes per core
P = 128
NBLK = (NLOC + P - 1) // P  # 49 local node blocks
NLOCP = NBLK * P            # 6272
CS = 512                    # slots per PSUM chunk
WS = 8.0                    # fp8 weight scale
SEG = 8192                  # slots per SBUF segment (g/s tile)
EVAC_ACT = 4                # of every 10 pair-slots (2 chunks each)                # of every 10 chunks, this many ACT-evac'd

_prog_cache = {}


# --------------------------------------------------------------------------- host


def _even_runs(Kr):
    """Make every equal-k run even-length by bumping the first node of the
    following run up to k.  Returns (Kr', ghost) where ghost>0 means one
    ghost node with `ghost` slots is appended to keep the tail even."""
    Kr = Kr.copy()
    r = 0
    nloc = len(Kr)
    while r < nloc:
        k = int(Kr[r])
        r2 = r
        while r2 < nloc and Kr[r2] == k:
            r2 += 1
        if (r2 - r) % 2 == 1:
            if r2 < nloc:
                Kr[r2] = k     # borrow: run becomes [r, r2] (even)
                r = r2 + 1
                continue
            return Kr, k       # odd tail: ghost node with k slots
        r = r2
    return Kr, 0


def _schedule(Kr, ghost):
    """Compile-time layout shared by all cores.

    Returns (subruns, segments, tot):
      subruns: (k, rank0, n_nodes, slot0) — slot-major [k, n] layout
      segments: (slot0, slot_len, [subrun indices])
    """
    ranks = list(Kr) + ([ghost] if ghost else [])
    nr = len(ranks)
    subruns = []
    r = 0
    slot = 0
    while r < nr:
        k = int(ranks[r])
        r2 = r
        while r2 < nr and ranks[r2] == k:
            r2 += 1
        nmax = max(2, (SEG // k) & ~1)   # even node count per sub-run
        while r < r2:
            n = min(nmax, r2 - r)
            subruns.append((k, r, n, slot))
            slot += n * k
            r += n
    tot = slot

    segments = []
    cur, s0, cur_len = [], 0, 0
    for i, (k, r0, n, sl0) in enumerate(subruns):
        ln = n * k
        if cur and cur_len + ln > SEG:
            segments.append((s0, cur_len, cur))
            cur, s0, cur_len = [], sl0, 0
        cur.append(i)
        cur_len += ln
    if cur:
        segments.append((s0, cur_len, cur))
    return subruns, segments, tot


def _preprocess(src, dst):
    """Shard edges by destination, degree-sort nodes, build per-core slots."""
    src = np.asarray(src).astype(np.int64)
    dst = np.asarray(dst).astype(np.int64)

    order = np.argsort(dst, kind="stable")
    dst_s = dst[order]
    src_s = src[order]
    bounds = np.searchsorted(dst_s, np.arange(NCORES + 1) * NLOC)

    cores = []
    degsort = np.zeros((NCORES, NLOC), np.int64)
    for c in range(NCORES):
        a, b = bounds[c], bounds[c + 1]
        ldst = dst_s[a:b] - c * NLOC
        lsrc = src_s[a:b]
        deg = np.bincount(ldst, minlength=NLOC)
        perm = np.argsort(-deg, kind="stable")
        degsort[c] = deg[perm]
        cores.append((lsrc, deg, perm))

    assert degsort.min() >= 1, "deg-0 node: pad-slot scheme needs in-edges"
    Kr = degsort.max(axis=0)
    Kr, ghost = _even_runs(Kr)
    subruns, segments, tot = _schedule(Kr, ghost)

    # slot-major slot -> (rank, within) maps
    node_of_slot = np.empty(tot, np.int64)
    within = np.empty(tot, np.int64)
    for (k, r0, n, sl0) in subruns:
        node_of_slot[sl0 : sl0 + n * k] = r0 + np.tile(np.arange(n), k)
        within[sl0 : sl0 + n * k] = np.repeat(np.arange(k), n)

    per_core = []
    for c in range(NCORES):
        lsrc, deg, perm = cores[c]
        starts = np.concatenate([[0], np.cumsum(deg)])
        # ghost rank (if any) maps to node 0 (junk, overwritten by memset)
        perm_ext = np.concatenate([perm, [0]])
        node_ids = perm_ext[node_of_slot]
        eidx = starts[node_ids] + np.minimum(within, deg[node_ids] - 1)
        src_ids = lsrc[eidx]
        dst_ids = c * NLOC + node_ids
        per_core.append((src_ids, dst_ids, perm))
    return Kr, ghost, per_core


def _host_inputs(inputs, Kr, ghost, per_core):
    h = np.asarray(inputs["h"], np.float32)
    A_w = np.asarray(inputs["A_w"], np.float32)
    A_b = np.asarray(inputs["A_b"], np.float32)
    B_w = np.asarray(inputs["B_w"], np.float32)
    B_b = np.asarray(inputs["B_b"], np.float32)
    U_w = np.asarray(inputs["U_w"], np.float32)
    U_b = np.asarray(inputs["U_b"], np.float32)

    h8 = h.astype(F8)
    tot = int(Kr.sum()) + ghost

    b8 = (WS * B_w).astype(F8)
    bdr = np.ascontiguousarray(np.stack([b8, b8], axis=1).reshape(D, 2 * D))
    a8 = (WS * A_w).astype(F8)
    adr = np.ascontiguousarray(
        np.stack([a8, np.zeros_like(a8)], axis=1).reshape(D, 2 * D)
    )
    u1 = np.ascontiguousarray(U_w[:D]).astype(BF16)
    u2 = np.ascontiguousarray(U_w[D:] / WS).astype(BF16)
    ubr = U_b[None, :].astype(BF16)                   # [1, 128]
    twobb = (2.0 * B_b)[:, None].astype(np.float32)   # [128, 1]
    ab8 = (WS * A_b)[:, None].astype(np.float32)      # [128, 1]

    in_maps = []
    for c in range(NCORES):
        src_ids, dst_ids, perm = per_core[c]
        m = np.empty((tot, 2, D), F8)
        m[:, 0] = h8[src_ids]
        m[:, 1] = h8[dst_ids]
        hz = np.ascontiguousarray(m.reshape(2 * tot, D).T)  # [128, 2*TOT]

        hl = np.zeros((NLOCP, D), np.float32)
        hl[:NLOC] = h[c * NLOC + perm]
        hl_pm = np.ascontiguousarray(
            hl.reshape(NBLK, P, D).transpose(1, 0, 2).reshape(P, NBLK * D)
        ).astype(BF16)
        in_maps.append(
            {
                "hz": hz,
                "hloc": hl_pm,
                "hTloc": np.ascontiguousarray(hl.T).astype(BF16),
                "bdr": bdr,
                "adr": adr,
                "u1": u1,
                "u2": u2,
                "ubr": ubr,
                "twobb": twobb,
                "ab8": ab8,
            }
        )
    return in_maps


# --------------------------------------------------------------------------- bass


def _build(Kr, ghost):
    f32 = mybir.dt.float32
    bf16 = mybir.dt.bfloat16
    fp8 = mybir.dt.float8e4
    ALU = mybir.AluOpType
    ACT = mybir.ActivationFunctionType
    DR = mybir.MatmulPerfMode.DoubleRow

    subruns, segments, tot = _schedule(Kr, ghost)

    nc = bacc.Bacc(
        "TRN2", target_bir_lowering=False, debug=False, num_devices=NCORES
    )
    hz = nc.declare_dram_parameter("hz", [D, 2 * tot], fp8, isOutput=False)
    hloc = nc.declare_dram_parameter("hloc", [P, NBLK * D], bf16, isOutput=False)
    hTloc = nc.declare_dram_parameter("hTloc", [D, NLOCP], bf16, isOutput=False)
    bdr = nc.declare_dram_parameter("bdr", [D, 2 * D], fp8, isOutput=False)
    adr = nc.declare_dram_parameter("adr", [D, 2 * D], fp8, isOutput=False)
    u1 = nc.declare_dram_parameter("u1", [D, D], bf16, isOutput=False)
    u2 = nc.declare_dram_parameter("u2", [D, D], bf16, isOutput=False)
    ubr = nc.declare_dram_parameter("ubr", [1, D], bf16, isOutput=False)
    twobb = nc.declare_dram_parameter("twobb", [D, 1], f32, isOutput=False)
    ab8 = nc.declare_dram_parameter("ab8", [D, 1], f32, isOutput=False)
    outp = nc.declare_dram_parameter("out", [P, NBLK * D], bf16, isOutput=True)

    with TileContext(nc) as tc:
        with (
            tc.tile_pool(name="const", bufs=1) as cpool,
            tc.tile_pool(name="hzp", bufs=3) as hzpool,
            tc.tile_pool(name="sg", bufs=3) as sgpool,
            tc.tile_pool(name="aev", bufs=2) as aevpool,
            tc.tile_pool(name="psz", bufs=2, space="PSUM") as pszpool,
            tc.tile_pool(name="psa", bufs=2, space="PSUM") as psapool,
        ):
            # ---- constants
            bdr_t = cpool.tile([D, 2 * D], fp8)
            nc.sync.dma_start(out=bdr_t[:], in_=bdr[:, :])
            adr_t = cpool.tile([D, 2 * D], fp8)
            nc.sync.dma_start(out=adr_t[:], in_=adr[:, :])
            u1_t = cpool.tile([D, D], bf16)
            nc.sync.dma_start(out=u1_t[:], in_=u1[:, :])
            u2_t = cpool.tile([D, D], bf16)
            nc.sync.dma_start(out=u2_t[:], in_=u2[:, :])
            ubr_t = cpool.tile([1, D], bf16)
            nc.sync.dma_start(out=ubr_t[:], in_=ubr[:, :])
            twobb_t = cpool.tile([D, 1], f32)
            nc.sync.dma_start(out=twobb_t[:], in_=twobb[:, :])
            ab8_t = cpool.tile([D, 1], f32)
            nc.sync.dma_start(out=ab8_t[:], in_=ab8[:, :])
            ones_bf = cpool.tile([1, P], bf16)
            nc.vector.memset(ones_bf[:], 1.0)

            # DMA for hloc/hTloc is deferred into the segment loop so the
            # first hz slab is at the head of the DMA queue (kills the ramp).
            hloc_sb = cpool.tile([P, NBLK * D], bf16)
            hTloc_sb = cpool.tile([D, NLOCP], bf16)

            ct_all = cpool.tile([D, NLOCP], bf16)
            bu_r = cpool.tile([P, NBLK * D], bf16)
            out_sb = cpool.tile([P, NBLK * D], bf16)
            ssq_all = cpool.tile([P, NBLK], f32)
            nrm_all = cpool.tile([P, NBLK], f32)
            rn_all = cpool.tile([P, NBLK], f32)
            sq_dump = cpool.tile([P, D], bf16)

            ranks_done = [0]       # reduce-complete watermark (ranks)
            blocks_done = [0]
            chunk_idx = [0]
            finals_done = [0]
            NORM_SPLIT = 40

            def emit_finals(b0, b1):
                nc.scalar.activation(
                    out=nrm_all[:, b0:b1], in_=ssq_all[:, b0:b1], func=ACT.Sqrt
                )
                nc.vector.reciprocal(rn_all[:, b0:b1], nrm_all[:, b0:b1])
                for b in range(b0, b1):
                    nc.vector.scalar_tensor_tensor(
                        out=out_sb[:, b * D : (b + 1) * D],
                        in0=bu_r[:, b * D : (b + 1) * D],
                        scalar=rn_all[:, b : b + 1],
                        in1=hloc_sb[:, b * D : (b + 1) * D],
                        op0=ALU.mult,
                        op1=ALU.add,
                    )

            def emit_bundle(b):
                bp_t = psapool.tile([P, 2 * CS], f32, tag="a", name="bp")
                bp = bp_t[:, :D]
                nc.tensor.matmul(
                    out=bp[:],
                    lhsT=hTloc_sb[:, b * P : (b + 1) * P],
                    rhs=u1_t[:],
                    start=True,
                    stop=False,
                )
                nc.tensor.matmul(
                    out=bp[:],
                    lhsT=ct_all[:, b * P : (b + 1) * P],
                    rhs=u2_t[:],
                    start=False,
                    stop=False,
                )
                nc.tensor.matmul(
                    out=bp[:], lhsT=ones_bf[:], rhs=ubr_t[:],
                    start=False, stop=True,
                )
                # ssq = sum(bundle^2) via ACT Square+accum (Square is in every
                # table set -> no reload); bu_r = relu(bundle) on ACT too.
                nc.scalar.activation(
                    out=sq_dump[:],
                    in_=bp[:],
                    func=ACT.Square,
                    accum_out=ssq_all[:, b : b + 1],
                )
                nc.scalar.activation(
                    out=bu_r[:, b * D : (b + 1) * D], in_=bp[:], func=ACT.Relu
                )

            def flush_bundles(limit):
                while (
                    blocks_done[0] < NBLK
                    and (blocks_done[0] + 1) * P <= limit
                ):
                    emit_bundle(blocks_done[0])
                    blocks_done[0] += 1

            # ---- main loop over segments.  Bundles are flushed with a
            # one-segment delay so their ct dependencies are already met by
            # the time the PE/ACT FIFOs reach them (no head-of-line stall).
            for seg_i, (s0, slen, sr_idx) in enumerate(segments):
                limit_entry = ranks_done[0]
                if seg_i == 1:
                    nc.sync.dma_start(out=hTloc_sb[:], in_=hTloc[:, :])
                elif seg_i == 2:
                    nc.sync.dma_start(out=hloc_sb[:], in_=hloc[:, :])
                hz_t = hzpool.tile([D, 2 * SEG], fp8, tag="hz")
                nc.sync.dma_start(
                    out=hz_t[:, : 2 * slen], in_=hz[:, 2 * s0 : 2 * (s0 + slen)]
                )
                sg_t = sgpool.tile([D, SEG], bf16, tag="sg")

                nch = (slen + CS - 1) // CS
                # groups of 4 chunks: [z z z z][a a a a] to amortize LDW
                for g0 in range(0, nch, 4):
                    gn = min(4, nch - g0)
                    spans = []
                    for ci in range(g0, g0 + gn):
                        c0 = ci * CS
                        spans.append((c0, min(CS, slen - c0)))
                    # z pairs
                    zps = []
                    for pi in range(0, gn, 2):
                        pair = spans[pi : pi + 2]
                        zp = pszpool.tile([P, 2 * CS], f32, tag="z")
                        for j, (c0, cs) in enumerate(pair):
                            nc.tensor.matmul(
                                out=zp[:, j * CS : j * CS + cs],
                                lhsT=bdr_t[:].rearrange("p (i m) -> p i m", i=2),
                                rhs=hz_t[:, 2 * c0 : 2 * (c0 + cs)].rearrange(
                                    "p (n i) -> p i n", i=2
                                ),
                                start=True,
                                stop=True,
                                perf_mode=DR,
                            )
                        zps.append((zp, pair))
                    # a chunks, paired into [128,1024] psum tiles
                    aps = []
                    for pi in range(0, gn, 2):
                        pair = spans[pi : pi + 2]
                        ap = psapool.tile([P, 2 * CS], f32, tag="a")
                        for j, (c0, cs) in enumerate(pair):
                            nc.tensor.matmul(
                                out=ap[:, j * CS : j * CS + cs],
                                lhsT=adr_t[:].rearrange("p (i m) -> p i m", i=2),
                                rhs=hz_t[:, 2 * c0 : 2 * (c0 + cs)].rearrange(
                                    "p (n i) -> p i n", i=2
                                ),
                                start=True,
                                stop=True,
                                perf_mode=DR,
                            )
                        aps.append((ap, pair))
                    # sigmoid over each z pair (1024 cols)
                    for zp, pair in zps:
                        pc0 = pair[0][0]
                        plen = sum(cs for _, cs in pair)
                        nc.scalar.activation(
                            out=sg_t[:, pc0 : pc0 + plen],
                            in_=zp[:, :plen],
                            func=ACT.Sigmoid,
                            bias=twobb_t[:, 0:1],
                            scale=1.0 / WS,
                        )
                    # g = (a + 8*A_b) * s, in place over s, per a-pair.
                    # GPSIMD cannot read PSUM, so evac is DVE stt (1x) or,
                    # for EVAC_ACT of 10 pairs, ACT Identity+bias then a DVE
                    # 2x tensor_tensor multiply.
                    for ap, pair in aps:
                        pc0 = pair[0][0]
                        plen = sum(cs for _, cs in pair)
                        if chunk_idx[0] % 10 < EVAC_ACT:
                            ae_t = aevpool.tile([P, 2 * CS], bf16, tag="ae")
                            nc.scalar.activation(
                                out=ae_t[:, :plen],
                                in_=ap[:, :plen],
                                func=ACT.Identity,
                                bias=ab8_t[:, 0:1],
                            )
                            nc.vector.tensor_tensor(
                                out=sg_t[:, pc0 : pc0 + plen],
                                in0=ae_t[:, :plen],
                                in1=sg_t[:, pc0 : pc0 + plen],
                                op=ALU.mult,
                            )
                        else:
                            nc.vector.scalar_tensor_tensor(
                                out=sg_t[:, pc0 : pc0 + plen],
                                in0=ap[:, :plen],
                                scalar=ab8_t[:, 0:1],
                                in1=sg_t[:, pc0 : pc0 + plen],
                                op0=ALU.add,
                                op1=ALU.mult,
                            )
                        chunk_idx[0] += 2

                # segment-max per sub-run: slot-major flat fold tree.
                # Each fold combines slot-rows [0,h) with [cur-h,cur) as flat
                # contiguous ranges (always 4B-aligned for even n -> DVE 2x);
                # overlap is safe because max is idempotent.  The final fold
                # (cur==2) writes ct directly; clamp >= 0 via 4x tensor_scalar.
                for i in sr_idx:
                    k, r0, n, sl0 = subruns[i]
                    off = sl0 - s0
                    flat = sg_t[:, off : off + n * k]
                    cur = k
                    while cur > 2:
                        h = (cur + 1) // 2
                        nc.vector.tensor_tensor(
                            out=flat[:, 0 : h * n],
                            in0=flat[:, 0 : h * n],
                            in1=flat[:, (cur - h) * n : cur * n],
                            op=ALU.max,
                        )
                        cur = h
                    nc.vector.tensor_tensor(
                        out=ct_all[:, r0 : r0 + n],
                        in0=flat[:, 0:n],
                        in1=flat[:, n : 2 * n],
                        op=ALU.max,
                    )
                    nc.vector.tensor_scalar_max(
                        ct_all[:, r0 : r0 + n], ct_all[:, r0 : r0 + n], 0.0
                    )
                    ranks_done[0] = min(r0 + n, NLOC)
                flush_bundles(limit_entry)
                if blocks_done[0] >= NORM_SPLIT and finals_done[0] == 0:
                    emit_finals(0, NORM_SPLIT)
                    finals_done[0] = NORM_SPLIT

            # pad ranks (incl. ghost) -> ct = 0, then remaining bundles
            if NLOCP > NLOC:
                nc.vector.memset(ct_all[:, NLOC:], 0.0)
            flush_bundles(NLOCP)

            # ---- remaining norm + final combine
            emit_finals(finals_done[0], NBLK)
            nc.sync.dma_start(out=outp[:, :], in_=out_sb[:])

    nc.compile()
    return nc


# --------------------------------------------------------------------------- run


def _run(inputs, trace=False):
    Kr, ghost, per_core = _preprocess(inputs["src"], inputs["dst"])

    key = (ghost,) + tuple(int(k) for k in Kr)
    if key not in _prog_cache:
        _prog_cache.clear()
        _prog_cache[key] = _build(Kr, ghost)
    nc = _prog_cache[key]

    in_maps = _host_inputs(inputs, Kr, ghost, per_core)
    res = run_bass_kernel_spmd(nc, in_maps, list(range(NCORES)), trace=trace)

    out = np.empty((N, D), np.float32)
    for c in range(NCORES):
        _, _, perm = per_core[c]
        o_pm = np.asarray(res.results[c]["out"]).astype(np.float32)
        o = o_pm.reshape(P, NBLK, D).transpose(1, 0, 2).reshape(NLOCP, D)
        out[c * NLOC + perm] = o[:NLOC]
    return out, res


def kernel(**inputs) -> np.ndarray:
    out, _ = _run(inputs, trace=False)
    return out


# revision 21
# speedup vs baseline: 1.1411x; 1.1411x over previous
"""GraphSAGE edge layer on 8 Trainium2 NeuronCores via Bass/Tile.  v4.3.

Gather-free, feature-major design (device cannot gather rows by edge on this
image): the HOST replicates edge endpoints' raw features into dense slot
streams (sharding/halo distribution on the host, all model FLOPs on device).

v4.3 design notes (driven by v3/v4.2 traces):
  - Slot layout: per-core nodes are degree-sorted; rank r gets
    Kr[r] = max over cores of the r-th sorted degree (shared across cores ->
    one SPMD program; ~1% padding).  Equal-k runs are made even-length by
    bumping one node into the previous (larger-k) run; an optional ghost
    node keeps the tail even.
  - Within each sub-run the slots are SLOT-MAJOR ([k, n]: slot j of all n
    nodes contiguous), so the segment-max tree folds are flat contiguous
    tensor_tensor max ops that hit the DVE 2x packed mode, and the final
    fold writes ct directly.
  - Gate pipeline per 512-slot chunk, weights grouped [z z z z][a a a a]:
      z-psum pair [128,1024] <- 2 DoubleRow fp8 matmuls (contract 256)
      a-psum [128,512]       <- fp8 matmul of 8*A_w against stride-2 src
      s = ACT sigmoid(zpair/8 + 2*B_b) over the 1024-col pair
      g = (a + 8*A_b) * s in place over s: DVE stt (1x, PSUM-bound), with a
        fraction of chunks evacuated by ACT (Identity+bias) so the DVE
        multiply runs as 2x tensor_tensor (EVAC_ACT tunable).
  - Update per 128-node block: one PSUM tile accumulates h@U1 + ct.T@(U2/8)
    + ones.T@U_b (3 matmuls); ACT Relu -> bu_r; ACT Square+accum -> ssq.
    After all blocks: one Sqrt + reciprocal, then per block a single DVE
    stt: out = bu_r * rsqrt(ssq) + h (bf16).  GPSIMD is not used: it cannot
    read PSUM and measured ~7x slower than modeled on tensor ops.
  - hloc/out shipped as bf16 (residual precision ~0.4%, gate 2e-2).

Precision: ~2e-3 rel err end-to-end vs the 2e-2 gate.
"""

import numpy as np
import ml_dtypes

from concourse import bass, bacc, mybir
from concourse.tile import TileContext
from concourse.bass_utils import run_bass_kernel_spmd

BF16 = ml_dtypes.bfloat16
F8 = ml_dtypes.float8_e4m3

N = 50000
E = 800000
D = 128
NCORES = 8
NLOC = N // NCORES          # 6250 nodes per core
P = 128
NBLK = (NLOC + P - 1) // P  # 49 local node blocks
NLOCP = NBLK * P            # 6272
CS = 512                    # slots per PSUM chunk
WS = 8.0                    # fp8 weight scale
SEG = 8192                  # slots per SBUF segment (g/s tile)
EVAC_ACT = 4                # of every 10 chunks, this many ACT-evac'd

_prog_cache = {}


# --------------------------------------------------------------------------- host


def _even_runs(Kr):
    """Make every equal-k run even-length by bumping the first node of the
    following run up to k.  Returns (Kr', ghost) where ghost>0 means one
    ghost node with `ghost` slots is appended to keep the tail even."""
    Kr = Kr.copy()
    r = 0
    nloc = len(Kr)
    while r < nloc:
        k = int(Kr[r])
        r2 = r
        while r2 < nloc and Kr[r2] == k:
            r2 += 1
        if (r2 - r) % 2 == 1:
            if r2 < nloc:
                Kr[r2] = k     # borrow: run becomes [r, r2] (even)
                r = r2 + 1
                continue
            return Kr, k       # odd tail: ghost node with k slots
        r = r2
    return Kr, 0


def _schedule(Kr, ghost):
    """Compile-time layout shared by all cores.

    Returns (subruns, segments, tot):
      subruns: (k, rank0, n_nodes, slot0) — slot-major [k, n] layout
      segments: (slot0, slot_len, [subrun indices])
    """
    ranks = list(Kr) + ([ghost] if ghost else [])
    nr = len(ranks)
    subruns = []
    r = 0
    slot = 0
    while r < nr:
        k = int(ranks[r])
        r2 = r
        while r2 < nr and ranks[r2] == k:
            r2 += 1
        nmax = max(2, (SEG // k) & ~1)   # even node count per sub-run
        while r < r2:
            n = min(nmax, r2 - r)
            subruns.append((k, r, n, slot))
            slot += n * k
            r += n
    tot = slot

    segments = []
    cur, s0, cur_len = [], 0, 0
    for i, (k, r0, n, sl0) in enumerate(subruns):
        ln = n * k
        # graded first segments so the pipeline fills quickly
        cap = 2048 if len(segments) == 0 else (4096 if len(segments) == 1 else SEG)
        if cur and cur_len + ln > cap:
            segments.append((s0, cur_len, cur))
            cur, s0, cur_len = [], sl0, 0
        cur.append(i)
        cur_len += ln
    if cur:
        segments.append((s0, cur_len, cur))
    return subruns, segments, tot


def _preprocess(src, dst):
    """Shard edges by destination, degree-sort nodes, build per-core slots."""
    src = np.asarray(src).astype(np.int64)
    dst = np.asarray(dst).astype(np.int64)

    order = np.argsort(dst, kind="stable")
    dst_s = dst[order]
    src_s = src[order]
    bounds = np.searchsorted(dst_s, np.arange(NCORES + 1) * NLOC)

    cores = []
    degsort = np.zeros((NCORES, NLOC), np.int64)
    for c in range(NCORES):
        a, b = bounds[c], bounds[c + 1]
        ldst = dst_s[a:b] - c * NLOC
        lsrc = src_s[a:b]
        deg = np.bincount(ldst, minlength=NLOC)
        perm = np.argsort(-deg, kind="stable")
        degsort[c] = deg[perm]
        cores.append((lsrc, deg, perm))

    assert degsort.min() >= 1, "deg-0 node: pad-slot scheme needs in-edges"
    Kr = degsort.max(axis=0)
    Kr, ghost = _even_runs(Kr)
    subruns, segments, tot = _schedule(Kr, ghost)

    # slot-major slot -> (rank, within) maps
    node_of_slot = np.empty(tot, np.int64)
    within = np.empty(tot, np.int64)
    for (k, r0, n, sl0) in subruns:
        node_of_slot[sl0 : sl0 + n * k] = r0 + np.tile(np.arange(n), k)
        within[sl0 : sl0 + n * k] = np.repeat(np.arange(k), n)

    per_core = []
    for c in range(NCORES):
        lsrc, deg, perm = cores[c]
        starts = np.concatenate([[0], np.cumsum(deg)])
        # ghost rank (if any) maps to node 0 (junk, overwritten by memset)
        perm_ext = np.concatenate([perm, [0]])
        node_ids = perm_ext[node_of_slot]
        eidx = starts[node_ids] + np.minimum(within, deg[node_ids] - 1)
        src_ids = lsrc[eidx]
        dst_ids = c * NLOC + node_ids
        per_core.append((src_ids, dst_ids, perm))
    return Kr, ghost, per_core


def _host_inputs(inputs, Kr, ghost, per_core):
    h = np.asarray(inputs["h"], np.float32)
    A_w = np.asarray(inputs["A_w"], np.float32)
    A_b = np.asarray(inputs["A_b"], np.float32)
    B_w = np.asarray(inputs["B_w"], np.float32)
    B_b = np.asarray(inputs["B_b"], np.float32)
    U_w = np.asarray(inputs["U_w"], np.float32)
    U_b = np.asarray(inputs["U_b"], np.float32)

    h8 = h.astype(F8)
    tot = int(Kr.sum()) + ghost

    b8 = (WS * B_w).astype(F8)
    bdr = np.ascontiguousarray(np.stack([b8, b8], axis=1).reshape(D, 2 * D))
    a8 = (WS * A_w).astype(F8)
    adr = np.ascontiguousarray(
        np.stack([a8, np.zeros_like(a8)], axis=1).reshape(D, 2 * D)
    )
    u1 = np.ascontiguousarray(U_w[:D]).astype(BF16)
    u2 = np.ascontiguousarray(U_w[D:] / WS).astype(BF16)
    ubr = U_b[None, :].astype(BF16)                   # [1, 128]
    twobb = (2.0 * B_b)[:, None].astype(np.float32)   # [128, 1]
    ab8 = (WS * A_b)[:, None].astype(np.float32)      # [128, 1]

    in_maps = []
    for c in range(NCORES):
        src_ids, dst_ids, perm = per_core[c]
        m = np.empty((tot, 2, D), F8)
        m[:, 0] = h8[src_ids]
        m[:, 1] = h8[dst_ids]
        hz = np.ascontiguousarray(m.reshape(2 * tot, D).T)  # [128, 2*TOT]

        hl = np.zeros((NLOCP, D), np.float32)
        hl[:NLOC] = h[c * NLOC + perm]
        hl_pm = np.ascontiguousarray(
            hl.reshape(NBLK, P, D).transpose(1, 0, 2).reshape(P, NBLK * D)
        ).astype(BF16)
        in_maps.append(
            {
                "hz": hz,
                "hloc": hl_pm,
                "hTloc": np.ascontiguousarray(hl.T).astype(BF16),
                "bdr": bdr,
                "adr": adr,
                "u1": u1,
                "u2": u2,
                "ubr": ubr,
                "twobb": twobb,
                "ab8": ab8,
            }
        )
    return in_maps


# --------------------------------------------------------------------------- bass


def _build(Kr, ghost):
    f32 = mybir.dt.float32
    bf16 = mybir.dt.bfloat16
    fp8 = mybir.dt.float8e4
    ALU = mybir.AluOpType
    ACT = mybir.ActivationFunctionType
    DR = mybir.MatmulPerfMode.DoubleRow

    subruns, segments, tot = _schedule(Kr, ghost)

    nc = bacc.Bacc(
        "TRN2", target_bir_lowering=False, debug=False, num_devices=NCORES
    )
    hz = nc.declare_dram_parameter("hz", [D, 2 * tot], fp8, isOutput=False)
    hloc = nc.declare_dram_parameter("hloc", [P, NBLK * D], bf16, isOutput=False)
    hTloc = nc.declare_dram_parameter("hTloc", [D, NLOCP], bf16, isOutput=False)
    bdr = nc.declare_dram_parameter("bdr", [D, 2 * D], fp8, isOutput=False)
    adr = nc.declare_dram_parameter("adr", [D, 2 * D], fp8, isOutput=False)
    u1 = nc.declare_dram_parameter("u1", [D, D], bf16, isOutput=False)
    u2 = nc.declare_dram_parameter("u2", [D, D], bf16, isOutput=False)
    ubr = nc.declare_dram_parameter("ubr", [1, D], bf16, isOutput=False)
    twobb = nc.declare_dram_parameter("twobb", [D, 1], f32, isOutput=False)
    ab8 = nc.declare_dram_parameter("ab8", [D, 1], f32, isOutput=False)
    outp = nc.declare_dram_parameter("out", [P, NBLK * D], bf16, isOutput=True)

    with TileContext(nc) as tc:
        with (
            tc.tile_pool(name="const", bufs=1) as cpool,
            tc.tile_pool(name="hzp", bufs=3) as hzpool,
            tc.tile_pool(name="sg", bufs=3) as sgpool,
            tc.tile_pool(name="aev", bufs=2) as aevpool,
            tc.tile_pool(name="psz", bufs=2, space="PSUM") as pszpool,
            tc.tile_pool(name="psa", bufs=2, space="PSUM") as psapool,
        ):
            # ---- segment-0 hz slab first: heads the DMA queue so PE can
            # start ~10us earlier (consts are tiny and follow immediately)
            s0_first, slen_first, _ = segments[0]
            hz0_t = hzpool.tile([D, 2 * SEG], fp8, tag="hz")
            nc.sync.dma_start(
                out=hz0_t[:, : 2 * slen_first],
                in_=hz[:, 2 * s0_first : 2 * (s0_first + slen_first)],
            )

            # ---- constants
            bdr_t = cpool.tile([D, 2 * D], fp8)
            nc.sync.dma_start(out=bdr_t[:], in_=bdr[:, :])
            adr_t = cpool.tile([D, 2 * D], fp8)
            nc.sync.dma_start(out=adr_t[:], in_=adr[:, :])
            u1_t = cpool.tile([D, D], bf16)
            nc.sync.dma_start(out=u1_t[:], in_=u1[:, :])
            u2_t = cpool.tile([D, D], bf16)
            nc.sync.dma_start(out=u2_t[:], in_=u2[:, :])
            ubr_t = cpool.tile([1, D], bf16)
            nc.sync.dma_start(out=ubr_t[:], in_=ubr[:, :])
            twobb_t = cpool.tile([D, 1], f32)
            nc.sync.dma_start(out=twobb_t[:], in_=twobb[:, :])
            ab8_t = cpool.tile([D, 1], f32)
            nc.sync.dma_start(out=ab8_t[:], in_=ab8[:, :])
            ones_bf = cpool.tile([1, P], bf16)
            nc.vector.memset(ones_bf[:], 1.0)

            # DMA for hloc/hTloc is deferred into the segment loop so the
            # first hz slab is at the head of the DMA queue (kills the ramp).
            hloc_sb = cpool.tile([P, NBLK * D], bf16)
            hTloc_sb = cpool.tile([D, NLOCP], bf16)

            ct_all = cpool.tile([D, NLOCP], bf16)
            bu_r = cpool.tile([P, NBLK * D], bf16)
            out_sb = cpool.tile([P, NBLK * D], bf16)
            ssq_all = cpool.tile([P, NBLK], f32)
            nrm_all = cpool.tile([P, NBLK], f32)
            rn_all = cpool.tile([P, NBLK], f32)
            sq_dump = cpool.tile([P, D], bf16)

            ranks_done = [0]       # reduce-complete watermark (ranks)
            blocks_done = [0]
            chunk_idx = [0]
            finals_done = [0]
            NORM_SPLIT = 40

            def emit_finals(b0, b1):
                nc.scalar.activation(
                    out=nrm_all[:, b0:b1], in_=ssq_all[:, b0:b1], func=ACT.Sqrt
                )
                nc.vector.reciprocal(rn_all[:, b0:b1], nrm_all[:, b0:b1])
                for b in range(b0, b1):
                    nc.vector.scalar_tensor_tensor(
                        out=out_sb[:, b * D : (b + 1) * D],
                        in0=bu_r[:, b * D : (b + 1) * D],
                        scalar=rn_all[:, b : b + 1],
                        in1=hloc_sb[:, b * D : (b + 1) * D],
                        op0=ALU.mult,
                        op1=ALU.add,
                    )

            def emit_bundle(b):
                bp_t = psapool.tile([P, 2 * CS], f32, tag="a", name="bp")
                bp = bp_t[:, :D]
                nc.tensor.matmul(
                    out=bp[:],
                    lhsT=hTloc_sb[:, b * P : (b + 1) * P],
                    rhs=u1_t[:],
                    start=True,
                    stop=False,
                )
                nc.tensor.matmul(
                    out=bp[:],
                    lhsT=ct_all[:, b * P : (b + 1) * P],
                    rhs=u2_t[:],
                    start=False,
                    stop=False,
                )
                nc.tensor.matmul(
                    out=bp[:], lhsT=ones_bf[:], rhs=ubr_t[:],
                    start=False, stop=True,
                )
                # ssq = sum(bundle^2) via ACT Square+accum (Square is in every
                # table set -> no reload); bu_r = relu(bundle) on ACT too.
                nc.scalar.activation(
                    out=sq_dump[:],
                    in_=bp[:],
                    func=ACT.Square,
                    accum_out=ssq_all[:, b : b + 1],
                )
                nc.scalar.activation(
                    out=bu_r[:, b * D : (b + 1) * D], in_=bp[:], func=ACT.Relu
                )

            def flush_bundles(limit):
                while (
                    blocks_done[0] < NBLK
                    and (blocks_done[0] + 1) * P <= limit
                ):
                    emit_bundle(blocks_done[0])
                    blocks_done[0] += 1

            # ---- main loop over segments.  Bundles are flushed with a
            # one-segment delay so their ct dependencies are already met by
            # the time the PE/ACT FIFOs reach them (no head-of-line stall).
            for seg_i, (s0, slen, sr_idx) in enumerate(segments):
                limit_entry = ranks_done[0]
                if seg_i == 1:
                    nc.sync.dma_start(out=hTloc_sb[:], in_=hTloc[:, :])
                elif seg_i == 2:
                    nc.sync.dma_start(out=hloc_sb[:], in_=hloc[:, :])
                if seg_i == 0:
                    hz_t = hz0_t
                else:
                    hz_t = hzpool.tile([D, 2 * SEG], fp8, tag="hz")
                    nc.sync.dma_start(
                        out=hz_t[:, : 2 * slen],
                        in_=hz[:, 2 * s0 : 2 * (s0 + slen)],
                    )
                sg_t = sgpool.tile([D, SEG], bf16, tag="sg")

                nch = (slen + CS - 1) // CS
                # groups of 4 chunks: [z z z z][a a a a] to amortize LDW
                for g0 in range(0, nch, 4):
                    gn = min(4, nch - g0)
                    spans = []
                    for ci in range(g0, g0 + gn):
                        c0 = ci * CS
                        spans.append((c0, min(CS, slen - c0)))
                    # z pairs
                    zps = []
                    for pi in range(0, gn, 2):
                        pair = spans[pi : pi + 2]
                        zp = pszpool.tile([P, 2 * CS], f32, tag="z")
                        for j, (c0, cs) in enumerate(pair):
                            nc.tensor.matmul(
                                out=zp[:, j * CS : j * CS + cs],
                                lhsT=bdr_t[:].rearrange("p (i m) -> p i m", i=2),
                                rhs=hz_t[:, 2 * c0 : 2 * (c0 + cs)].rearrange(
                                    "p (n i) -> p i n", i=2
                                ),
                                start=True,
                                stop=True,
                                perf_mode=DR,
                            )
                        zps.append((zp, pair))
                    # a chunks, paired into [128,1024] psum tiles
                    aps = []
                    for pi in range(0, gn, 2):
                        pair = spans[pi : pi + 2]
                        ap = psapool.tile([P, 2 * CS], f32, tag="a")
                        for j, (c0, cs) in enumerate(pair):
                            nc.tensor.matmul(
                                out=ap[:, j * CS : j * CS + cs],
                                lhsT=adr_t[:].rearrange("p (i m) -> p i m", i=2),
                                rhs=hz_t[:, 2 * c0 : 2 * (c0 + cs)].rearrange(
                                    "p (n i) -> p i n", i=2
                                ),
                                start=True,
                                stop=True,
                                perf_mode=DR,
                            )
                        aps.append((ap, pair))
                    # sigmoid over each z pair (1024 cols)
                    for zp, pair in zps:
                        pc0 = pair[0][0]
                        plen = sum(cs for _, cs in pair)
                        nc.scalar.activation(
                            out=sg_t[:, pc0 : pc0 + plen],
                            in_=zp[:, :plen],
                            func=ACT.Sigmoid,
                            bias=twobb_t[:, 0:1],
                            scale=1.0 / WS,
                        )
                    # g = (a + 8*A_b) * s, in place over s, per a-pair.
                    # GPSIMD cannot read PSUM, so evac is DVE stt (1x) or,
                    # for EVAC_ACT of 10 pairs, ACT Identity+bias then a DVE
                    # 2x tensor_tensor multiply.
                    for ap, pair in aps:
                        pc0 = pair[0][0]
                        plen = sum(cs for _, cs in pair)
                        if chunk_idx[0] % 10 < EVAC_ACT:
                            ae_t = aevpool.tile([P, 2 * CS], bf16, tag="ae")
                            nc.scalar.activation(
                                out=ae_t[:, :plen],
                                in_=ap[:, :plen],
                                func=ACT.Identity,
                                bias=ab8_t[:, 0:1],
                            )
                            nc.vector.tensor_tensor(
                                out=sg_t[:, pc0 : pc0 + plen],
                                in0=ae_t[:, :plen],
                                in1=sg_t[:, pc0 : pc0 + plen],
                                op=ALU.mult,
                            )
                        else:
                            nc.vector.scalar_tensor_tensor(
                                out=sg_t[:, pc0 : pc0 + plen],
                                in0=ap[:, :plen],
                                scalar=ab8_t[:, 0:1],
                                in1=sg_t[:, pc0 : pc0 + plen],
                                op0=ALU.add,
                                op1=ALU.mult,
                            )
                        chunk_idx[0] += 2

                # segment-max per sub-run: slot-major flat fold tree.
                # Each fold combines slot-rows [0,h) with [cur-h,cur) as flat
                # contiguous ranges (always 4B-aligned for even n -> DVE 2x);
                # overlap is safe because max is idempotent.  The final fold
                # (cur==2) writes ct directly; clamp >= 0 via 4x tensor_scalar.
                for i in sr_idx:
                    k, r0, n, sl0 = subruns[i]
                    off = sl0 - s0
                    flat = sg_t[:, off : off + n * k]
                    cur = k
                    while cur > 2:
                        h = (cur + 1) // 2
                        nc.vector.tensor_tensor(
                            out=flat[:, 0 : h * n],
                            in0=flat[:, 0 : h * n],
                            in1=flat[:, (cur - h) * n : cur * n],
                            op=ALU.max,
                        )
                        cur = h
                    nc.vector.tensor_tensor(
                        out=ct_all[:, r0 : r0 + n],
                        in0=flat[:, 0:n],
                        in1=flat[:, n : 2 * n],
                        op=ALU.max,
                    )
                    nc.vector.tensor_scalar_max(
                        ct_all[:, r0 : r0 + n], ct_all[:, r0 : r0 + n], 0.0
                    )
                    ranks_done[0] = min(r0 + n, NLOC)
                flush_bundles(limit_entry)
                if blocks_done[0] >= NORM_SPLIT and finals_done[0] == 0:
                    emit_finals(0, NORM_SPLIT)
                    nc.sync.dma_start(
                        out=outp[:, : NORM_SPLIT * D],
                        in_=out_sb[:, : NORM_SPLIT * D],
                    )
                    finals_done[0] = NORM_SPLIT

            # pad ranks (incl. ghost) -> ct = 0, then remaining bundles
            if NLOCP > NLOC:
                nc.vector.memset(ct_all[:, NLOC:], 0.0)
            flush_bundles(NLOCP)

            # ---- remaining norm + final combine
            b0 = finals_done[0]
            emit_finals(b0, NBLK)
            nc.sync.dma_start(
                out=outp[:, b0 * D :], in_=out_sb[:, b0 * D :]
            )

    nc.compile()
    return nc


# --------------------------------------------------------------------------- run


def _run(inputs, trace=False):
    Kr, ghost, per_core = _preprocess(inputs["src"], inputs["dst"])

    key = (ghost,) + tuple(int(k) for k in Kr)
    if key not in _prog_cache:
        _prog_cache.clear()
        _prog_cache[key] = _build(Kr, ghost)
    nc = _prog_cache[key]

    in_maps = _host_inputs(inputs, Kr, ghost, per_core)
    res = run_bass_kernel_spmd(nc, in_maps, list(range(NCORES)), trace=trace)

    out = np.empty((N, D), np.float32)
    for c in range(NCORES):
        _, _, perm = per_core[c]
        o_pm = np.asarray(res.results[c]["out"]).astype(np.float32)
        o = o_pm.reshape(P, NBLK, D).transpose(1, 0, 2).reshape(NLOCP, D)
        out[c * NLOC + perm] = o[:NLOC]
    return out, res


def kernel(**inputs) -> np.ndarray:
    out, _ = _run(inputs, trace=False)
    return out


# revision 22
# speedup vs baseline: 1.1997x; 1.0513x over previous
"""GraphSAGE edge layer on 8 Trainium2 NeuronCores via Bass/Tile.  v4.3.

Gather-free, feature-major design (device cannot gather rows by edge on this
image): the HOST replicates edge endpoints' raw features into dense slot
streams (sharding/halo distribution on the host, all model FLOPs on device).

v4.3 design notes (driven by v3/v4.2 traces):
  - Slot layout: per-core nodes are degree-sorted; rank r gets
    Kr[r] = max over cores of the r-th sorted degree (shared across cores ->
    one SPMD program; ~1% padding).  Equal-k runs are made even-length by
    bumping one node into the previous (larger-k) run; an optional ghost
    node keeps the tail even.
  - Within each sub-run the slots are SLOT-MAJOR ([k, n]: slot j of all n
    nodes contiguous), so the segment-max tree folds are flat contiguous
    tensor_tensor max ops that hit the DVE 2x packed mode, and the final
    fold writes ct directly.
  - Gate pipeline per 512-slot chunk, weights grouped [z z z z][a a a a]:
      z-psum pair [128,1024] <- 2 DoubleRow fp8 matmuls (contract 256)
      a-psum [128,512]       <- fp8 matmul of 8*A_w against stride-2 src
      s = ACT sigmoid(zpair/8 + 2*B_b) over the 1024-col pair
      g = (a + 8*A_b) * s in place over s: DVE stt (1x, PSUM-bound), with a
        fraction of chunks evacuated by ACT (Identity+bias) so the DVE
        multiply runs as 2x tensor_tensor (EVAC_ACT tunable).
  - Update per 128-node block: one PSUM tile accumulates h@U1 + ct.T@(U2/8)
    + ones.T@U_b (3 matmuls); ACT Relu -> bu_r; ACT Square+accum -> ssq.
    After all blocks: one Sqrt + reciprocal, then per block a single DVE
    stt: out = bu_r * rsqrt(ssq) + h (bf16).  GPSIMD is not used: it cannot
    read PSUM and measured ~7x slower than modeled on tensor ops.
  - hloc/out shipped as bf16 (residual precision ~0.4%, gate 2e-2).

Precision: ~2e-3 rel err end-to-end vs the 2e-2 gate.
"""

import numpy as np
import ml_dtypes

from concourse import bass, bacc, mybir
from concourse.tile import TileContext
from concourse.bass_utils import run_bass_kernel_spmd

BF16 = ml_dtypes.bfloat16
F8 = ml_dtypes.float8_e4m3

N = 50000
E = 800000
D = 128
NCORES = 8
NLOC = N // NCORES          # 6250 nodes per core
P = 128
NBLK = (NLOC + P - 1) // P  # 49 local node blocks
NLOCP = NBLK * P            # 6272
CS = 512                    # slots per PSUM chunk
WS = 8.0                    # fp8 weight scale
SEG = 8192                  # slots per SBUF segment (g/s tile)
EVAC_ACT = 4                # of every 10 chunks, this many ACT-evac'd

_prog_cache = {}


# --------------------------------------------------------------------------- host


def _even_runs(Kr):
    """Make every equal-k run even-length by bumping the first node of the
    following run up to k.  Returns (Kr', ghost) where ghost>0 means one
    ghost node with `ghost` slots is appended to keep the tail even."""
    Kr = Kr.copy()
    r = 0
    nloc = len(Kr)
    while r < nloc:
        k = int(Kr[r])
        r2 = r
        while r2 < nloc and Kr[r2] == k:
            r2 += 1
        if (r2 - r) % 2 == 1:
            if r2 < nloc:
                Kr[r2] = k     # borrow: run becomes [r, r2] (even)
                r = r2 + 1
                continue
            return Kr, k       # odd tail: ghost node with k slots
        r = r2
    return Kr, 0


def _schedule(Kr, ghost):
    """Compile-time layout shared by all cores.

    Returns (subruns, segments, tot):
      subruns: (k, rank0, n_nodes, slot0) — slot-major [k, n] layout
      segments: (slot0, slot_len, [subrun indices])
    """
    ranks = list(Kr) + ([ghost] if ghost else [])
    nr = len(ranks)
    subruns = []
    r = 0
    slot = 0
    while r < nr:
        k = int(ranks[r])
        r2 = r
        while r2 < nr and ranks[r2] == k:
            r2 += 1
        nmax = max(2, (SEG // k) & ~1)   # even node count per sub-run
        while r < r2:
            n = min(nmax, r2 - r)
            subruns.append((k, r, n, slot))
            slot += n * k
            r += n
    tot = slot

    segments = []
    cur, s0, cur_len = [], 0, 0
    for i, (k, r0, n, sl0) in enumerate(subruns):
        ln = n * k
        if cur and cur_len + ln > SEG:
            segments.append((s0, cur_len, cur))
            cur, s0, cur_len = [], sl0, 0
        cur.append(i)
        cur_len += ln
    if cur:
        segments.append((s0, cur_len, cur))
    return subruns, segments, tot


def _preprocess(src, dst):
    """Shard edges by destination, degree-sort nodes, build per-core slots."""
    src = np.asarray(src).astype(np.int64)
    dst = np.asarray(dst).astype(np.int64)

    order = np.argsort(dst, kind="stable")
    dst_s = dst[order]
    src_s = src[order]
    bounds = np.searchsorted(dst_s, np.arange(NCORES + 1) * NLOC)

    cores = []
    degsort = np.zeros((NCORES, NLOC), np.int64)
    for c in range(NCORES):
        a, b = bounds[c], bounds[c + 1]
        ldst = dst_s[a:b] - c * NLOC
        lsrc = src_s[a:b]
        deg = np.bincount(ldst, minlength=NLOC)
        perm = np.argsort(-deg, kind="stable")
        degsort[c] = deg[perm]
        cores.append((lsrc, deg, perm))

    assert degsort.min() >= 1, "deg-0 node: pad-slot scheme needs in-edges"
    Kr = degsort.max(axis=0)
    Kr, ghost = _even_runs(Kr)
    subruns, segments, tot = _schedule(Kr, ghost)

    # slot-major slot -> (rank, within) maps
    node_of_slot = np.empty(tot, np.int64)
    within = np.empty(tot, np.int64)
    for (k, r0, n, sl0) in subruns:
        node_of_slot[sl0 : sl0 + n * k] = r0 + np.tile(np.arange(n), k)
        within[sl0 : sl0 + n * k] = np.repeat(np.arange(k), n)

    per_core = []
    for c in range(NCORES):
        lsrc, deg, perm = cores[c]
        starts = np.concatenate([[0], np.cumsum(deg)])
        # ghost rank (if any) maps to node 0 (junk, overwritten by memset)
        perm_ext = np.concatenate([perm, [0]])
        node_ids = perm_ext[node_of_slot]
        eidx = starts[node_ids] + np.minimum(within, deg[node_ids] - 1)
        src_ids = lsrc[eidx]
        dst_ids = c * NLOC + node_ids
        per_core.append((src_ids, dst_ids, perm))
    return Kr, ghost, per_core


def _host_inputs(inputs, Kr, ghost, per_core):
    h = np.asarray(inputs["h"], np.float32)
    A_w = np.asarray(inputs["A_w"], np.float32)
    A_b = np.asarray(inputs["A_b"], np.float32)
    B_w = np.asarray(inputs["B_w"], np.float32)
    B_b = np.asarray(inputs["B_b"], np.float32)
    U_w = np.asarray(inputs["U_w"], np.float32)
    U_b = np.asarray(inputs["U_b"], np.float32)

    h8 = h.astype(F8)
    tot = int(Kr.sum()) + ghost

    b8 = (WS * B_w).astype(F8)
    bdr = np.ascontiguousarray(np.stack([b8, b8], axis=1).reshape(D, 2 * D))
    a8 = (WS * A_w).astype(F8)
    adr = np.ascontiguousarray(
        np.stack([a8, np.zeros_like(a8)], axis=1).reshape(D, 2 * D)
    )
    u1 = np.ascontiguousarray(U_w[:D]).astype(BF16)
    u2 = np.ascontiguousarray(U_w[D:] / WS).astype(BF16)
    ubr = U_b[None, :].astype(BF16)                   # [1, 128]
    twobb = (2.0 * B_b)[:, None].astype(np.float32)   # [128, 1]
    ab8 = (WS * A_b)[:, None].astype(np.float32)      # [128, 1]

    in_maps = []
    for c in range(NCORES):
        src_ids, dst_ids, perm = per_core[c]
        m = np.empty((tot, 2, D), F8)
        m[:, 0] = h8[src_ids]
        m[:, 1] = h8[dst_ids]
        hz = np.ascontiguousarray(m.reshape(2 * tot, D).T)  # [128, 2*TOT]

        hl = np.zeros((NLOCP, D), np.float32)
        hl[:NLOC] = h[c * NLOC + perm]
        hl_pm = np.ascontiguousarray(
            hl.reshape(NBLK, P, D).transpose(1, 0, 2).reshape(P, NBLK * D)
        ).astype(BF16)
        in_maps.append(
            {
                "hz": hz,
                "hloc": hl_pm,
                "hTloc": np.ascontiguousarray(hl.T).astype(BF16),
                "bdr": bdr,
                "adr": adr,
                "u1": u1,
                "u2": u2,
                "ubr": ubr,
                "twobb": twobb,
                "ab8": ab8,
            }
        )
    return in_maps


# --------------------------------------------------------------------------- bass


def _build(Kr, ghost):
    f32 = mybir.dt.float32
    bf16 = mybir.dt.bfloat16
    fp8 = mybir.dt.float8e4
    ALU = mybir.AluOpType
    ACT = mybir.ActivationFunctionType
    DR = mybir.MatmulPerfMode.DoubleRow

    subruns, segments, tot = _schedule(Kr, ghost)

    nc = bacc.Bacc(
        "TRN2", target_bir_lowering=False, debug=False, num_devices=NCORES
    )
    hz = nc.declare_dram_parameter("hz", [D, 2 * tot], fp8, isOutput=False)
    hloc = nc.declare_dram_parameter("hloc", [P, NBLK * D], bf16, isOutput=False)
    hTloc = nc.declare_dram_parameter("hTloc", [D, NLOCP], bf16, isOutput=False)
    bdr = nc.declare_dram_parameter("bdr", [D, 2 * D], fp8, isOutput=False)
    adr = nc.declare_dram_parameter("adr", [D, 2 * D], fp8, isOutput=False)
    u1 = nc.declare_dram_parameter("u1", [D, D], bf16, isOutput=False)
    u2 = nc.declare_dram_parameter("u2", [D, D], bf16, isOutput=False)
    ubr = nc.declare_dram_parameter("ubr", [1, D], bf16, isOutput=False)
    twobb = nc.declare_dram_parameter("twobb", [D, 1], f32, isOutput=False)
    ab8 = nc.declare_dram_parameter("ab8", [D, 1], f32, isOutput=False)
    outp = nc.declare_dram_parameter("out", [P, NBLK * D], bf16, isOutput=True)

    with TileContext(nc) as tc:
        with (
            tc.tile_pool(name="const", bufs=1) as cpool,
            tc.tile_pool(name="hzp", bufs=3) as hzpool,
            tc.tile_pool(name="sg", bufs=3) as sgpool,
            tc.tile_pool(name="aev", bufs=2) as aevpool,
            tc.tile_pool(name="psz", bufs=2, space="PSUM") as pszpool,
            tc.tile_pool(name="psa", bufs=2, space="PSUM") as psapool,
        ):
            # ---- segment-0 hz slab first: heads the DMA queue so PE can
            # start ~10us earlier (consts are tiny and follow immediately)
            s0_first, slen_first, _ = segments[0]
            hz0_t = hzpool.tile([D, 2 * SEG], fp8, tag="hz")
            nc.sync.dma_start(
                out=hz0_t[:, : 2 * slen_first],
                in_=hz[:, 2 * s0_first : 2 * (s0_first + slen_first)],
            )

            # ---- constants
            bdr_t = cpool.tile([D, 2 * D], fp8)
            nc.sync.dma_start(out=bdr_t[:], in_=bdr[:, :])
            adr_t = cpool.tile([D, 2 * D], fp8)
            nc.sync.dma_start(out=adr_t[:], in_=adr[:, :])
            u1_t = cpool.tile([D, D], bf16)
            nc.sync.dma_start(out=u1_t[:], in_=u1[:, :])
            u2_t = cpool.tile([D, D], bf16)
            nc.sync.dma_start(out=u2_t[:], in_=u2[:, :])
            ubr_t = cpool.tile([1, D], bf16)
            nc.sync.dma_start(out=ubr_t[:], in_=ubr[:, :])
            twobb_t = cpool.tile([D, 1], f32)
            nc.sync.dma_start(out=twobb_t[:], in_=twobb[:, :])
            ab8_t = cpool.tile([D, 1], f32)
            nc.sync.dma_start(out=ab8_t[:], in_=ab8[:, :])
            ones_bf = cpool.tile([1, P], bf16)
            nc.vector.memset(ones_bf[:], 1.0)

            # DMA for hloc/hTloc is deferred into the segment loop so the
            # first hz slab is at the head of the DMA queue (kills the ramp).
            hloc_sb = cpool.tile([P, NBLK * D], bf16)
            hTloc_sb = cpool.tile([D, NLOCP], bf16)

            ct_all = cpool.tile([D, NLOCP], bf16)
            bu_r = cpool.tile([P, NBLK * D], bf16)
            out_sb = cpool.tile([P, NBLK * D], bf16)
            ssq_all = cpool.tile([P, NBLK], f32)
            nrm_all = cpool.tile([P, NBLK], f32)
            rn_all = cpool.tile([P, NBLK], f32)
            sq_dump = cpool.tile([P, D], bf16)

            ranks_done = [0]       # reduce-complete watermark (ranks)
            blocks_done = [0]
            chunk_idx = [0]
            finals_done = [0]
            NORM_SPLIT = 40

            def emit_finals(b0, b1):
                nc.scalar.activation(
                    out=nrm_all[:, b0:b1], in_=ssq_all[:, b0:b1], func=ACT.Sqrt
                )
                nc.vector.reciprocal(rn_all[:, b0:b1], nrm_all[:, b0:b1])
                for b in range(b0, b1):
                    nc.vector.scalar_tensor_tensor(
                        out=out_sb[:, b * D : (b + 1) * D],
                        in0=bu_r[:, b * D : (b + 1) * D],
                        scalar=rn_all[:, b : b + 1],
                        in1=hloc_sb[:, b * D : (b + 1) * D],
                        op0=ALU.mult,
                        op1=ALU.add,
                    )

            def emit_bundle(b):
                bp_t = psapool.tile([P, 2 * CS], f32, tag="a", name="bp")
                bp = bp_t[:, :D]
                nc.tensor.matmul(
                    out=bp[:],
                    lhsT=hTloc_sb[:, b * P : (b + 1) * P],
                    rhs=u1_t[:],
                    start=True,
                    stop=False,
                )
                nc.tensor.matmul(
                    out=bp[:],
                    lhsT=ct_all[:, b * P : (b + 1) * P],
                    rhs=u2_t[:],
                    start=False,
                    stop=False,
                )
                nc.tensor.matmul(
                    out=bp[:], lhsT=ones_bf[:], rhs=ubr_t[:],
                    start=False, stop=True,
                )
                # ssq = sum(bundle^2) via ACT Square+accum (Square is in every
                # table set -> no reload); bu_r = relu(bundle) on ACT too.
                nc.scalar.activation(
                    out=sq_dump[:],
                    in_=bp[:],
                    func=ACT.Square,
                    accum_out=ssq_all[:, b : b + 1],
                )
                nc.scalar.activation(
                    out=bu_r[:, b * D : (b + 1) * D], in_=bp[:], func=ACT.Relu
                )

            def flush_bundles(limit):
                while (
                    blocks_done[0] < NBLK
                    and (blocks_done[0] + 1) * P <= limit
                ):
                    emit_bundle(blocks_done[0])
                    blocks_done[0] += 1

            # ---- main loop over segments.  Bundles are flushed with a
            # one-segment delay so their ct dependencies are already met by
            # the time the PE/ACT FIFOs reach them (no head-of-line stall).
            for seg_i, (s0, slen, sr_idx) in enumerate(segments):
                limit_entry = ranks_done[0]
                if seg_i == 1:
                    nc.sync.dma_start(out=hTloc_sb[:], in_=hTloc[:, :])
                elif seg_i == 2:
                    nc.sync.dma_start(out=hloc_sb[:], in_=hloc[:, :])
                if seg_i == 0:
                    hz_t = hz0_t
                else:
                    hz_t = hzpool.tile([D, 2 * SEG], fp8, tag="hz")
                    nc.sync.dma_start(
                        out=hz_t[:, : 2 * slen],
                        in_=hz[:, 2 * s0 : 2 * (s0 + slen)],
                    )
                sg_t = sgpool.tile([D, SEG], bf16, tag="sg")

                nch = (slen + CS - 1) // CS
                # groups of 4 chunks: [z z z z][a a a a] to amortize LDW
                for g0 in range(0, nch, 4):
                    gn = min(4, nch - g0)
                    spans = []
                    for ci in range(g0, g0 + gn):
                        c0 = ci * CS
                        spans.append((c0, min(CS, slen - c0)))
                    # z pairs
                    zps = []
                    for pi in range(0, gn, 2):
                        pair = spans[pi : pi + 2]
                        zp = pszpool.tile([P, 2 * CS], f32, tag="z")
                        for j, (c0, cs) in enumerate(pair):
                            nc.tensor.matmul(
                                out=zp[:, j * CS : j * CS + cs],
                                lhsT=bdr_t[:].rearrange("p (i m) -> p i m", i=2),
                                rhs=hz_t[:, 2 * c0 : 2 * (c0 + cs)].rearrange(
                                    "p (n i) -> p i n", i=2
                                ),
                                start=True,
                                stop=True,
                                perf_mode=DR,
                            )
                        zps.append((zp, pair))
                    # a chunks, paired into [128,1024] psum tiles
                    aps = []
                    for pi in range(0, gn, 2):
                        pair = spans[pi : pi + 2]
                        ap = psapool.tile([P, 2 * CS], f32, tag="a")
                        for j, (c0, cs) in enumerate(pair):
                            nc.tensor.matmul(
                                out=ap[:, j * CS : j * CS + cs],
                                lhsT=adr_t[:].rearrange("p (i m) -> p i m", i=2),
                                rhs=hz_t[:, 2 * c0 : 2 * (c0 + cs)].rearrange(
                                    "p (n i) -> p i n", i=2
                                ),
                                start=True,
                                stop=True,
                                perf_mode=DR,
                            )
                        aps.append((ap, pair))
                    # sigmoid over each z pair (1024 cols)
                    for zp, pair in zps:
                        pc0 = pair[0][0]
                        plen = sum(cs for _, cs in pair)
                        nc.scalar.activation(
                            out=sg_t[:, pc0 : pc0 + plen],
                            in_=zp[:, :plen],
                            func=ACT.Sigmoid,
                            bias=twobb_t[:, 0:1],
                            scale=1.0 / WS,
                        )
                    # g = (a + 8*A_b) * s, in place over s, per a-pair.
                    # GPSIMD cannot read PSUM, so evac is DVE stt (1x) or,
                    # for EVAC_ACT of 10 pairs, ACT Identity+bias then a DVE
                    # 2x tensor_tensor multiply.
                    for ap, pair in aps:
                        pc0 = pair[0][0]
                        plen = sum(cs for _, cs in pair)
                        if chunk_idx[0] % 10 < EVAC_ACT:
                            ae_t = aevpool.tile([P, 2 * CS], bf16, tag="ae")
                            nc.scalar.activation(
                                out=ae_t[:, :plen],
                                in_=ap[:, :plen],
                                func=ACT.Identity,
                                bias=ab8_t[:, 0:1],
                            )
                            nc.vector.tensor_tensor(
                                out=sg_t[:, pc0 : pc0 + plen],
                                in0=ae_t[:, :plen],
                                in1=sg_t[:, pc0 : pc0 + plen],
                                op=ALU.mult,
                            )
                        else:
                            nc.vector.scalar_tensor_tensor(
                                out=sg_t[:, pc0 : pc0 + plen],
                                in0=ap[:, :plen],
                                scalar=ab8_t[:, 0:1],
                                in1=sg_t[:, pc0 : pc0 + plen],
                                op0=ALU.add,
                                op1=ALU.mult,
                            )
                        chunk_idx[0] += 2

                # segment-max per sub-run: slot-major flat fold tree.
                # Each fold combines slot-rows [0,h) with [cur-h,cur) as flat
                # contiguous ranges (always 4B-aligned for even n -> DVE 2x);
                # overlap is safe because max is idempotent.  The final fold
                # (cur==2) writes ct directly; clamp >= 0 via 4x tensor_scalar.
                for i in sr_idx:
                    k, r0, n, sl0 = subruns[i]
                    off = sl0 - s0
                    flat = sg_t[:, off : off + n * k]
                    cur = k
                    while cur > 2:
                        h = (cur + 1) // 2
                        nc.vector.tensor_tensor(
                            out=flat[:, 0 : h * n],
                            in0=flat[:, 0 : h * n],
                            in1=flat[:, (cur - h) * n : cur * n],
                            op=ALU.max,
                        )
                        cur = h
                    nc.vector.tensor_tensor(
                        out=ct_all[:, r0 : r0 + n],
                        in0=flat[:, 0:n],
                        in1=flat[:, n : 2 * n],
                        op=ALU.max,
                    )
                    nc.vector.tensor_scalar_max(
                        ct_all[:, r0 : r0 + n], ct_all[:, r0 : r0 + n], 0.0
                    )
                    ranks_done[0] = min(r0 + n, NLOC)
                flush_bundles(limit_entry)
                if blocks_done[0] >= NORM_SPLIT and finals_done[0] == 0:
                    emit_finals(0, NORM_SPLIT)
                    nc.sync.dma_start(
                        out=outp[:, : NORM_SPLIT * D],
                        in_=out_sb[:, : NORM_SPLIT * D],
                    )
                    finals_done[0] = NORM_SPLIT

            # pad ranks (incl. ghost) -> ct = 0, then remaining bundles
            if NLOCP > NLOC:
                nc.vector.memset(ct_all[:, NLOC:], 0.0)
            flush_bundles(NLOCP)

            # ---- remaining norm + final combine
            b0 = finals_done[0]
            emit_finals(b0, NBLK)
            nc.sync.dma_start(
                out=outp[:, b0 * D :], in_=out_sb[:, b0 * D :]
            )

    nc.compile()
    return nc


# --------------------------------------------------------------------------- run


def _run(inputs, trace=False):
    Kr, ghost, per_core = _preprocess(inputs["src"], inputs["dst"])

    key = (ghost,) + tuple(int(k) for k in Kr)
    if key not in _prog_cache:
        _prog_cache.clear()
        _prog_cache[key] = _build(Kr, ghost)
    nc = _prog_cache[key]

    in_maps = _host_inputs(inputs, Kr, ghost, per_core)
    res = run_bass_kernel_spmd(nc, in_maps, list(range(NCORES)), trace=trace)

    out = np.empty((N, D), np.float32)
    for c in range(NCORES):
        _, _, perm = per_core[c]
        o_pm = np.asarray(res.results[c]["out"]).astype(np.float32)
        o = o_pm.reshape(P, NBLK, D).transpose(1, 0, 2).reshape(NLOCP, D)
        out[c * NLOC + perm] = o[:NLOC]
    return out, res


def kernel(**inputs) -> np.ndarray:
    out, _ = _run(inputs, trace=False)
    return out
